# revision 19
# baseline (speedup 1.0000x reference)
"""Trainium2 Bass kernel for the e3nn-style InterModule:
   out = Linear2( NormAct( Linear1(x) ) )  over irreps
     IN  [(512,0),(256,1),(128,2)]  dim 1920
     MID [(1024,0),(512,1),(256,2)] dim 3840
     OUT = IN
   N = 32768 nodes, data-parallel over 8 cores (4096 nodes/core).

v3 design (feature-major end-to-end, bf16):
  - Host pre-transposes x into de-interleaved feature-major tiles
    xt[128p, 15ft, 4096n] (bf16) and pre-transposes/prescales weights,
    so the device does NO transposes at all.
  - Linear1: stationary W1 tile [u,128v], moving xt [u, n] -> h^T in PSUM.
    Slice order: l1, l2 first; l0 last — so the l1/l2 norm chain
    (squares/adds/sqrt) runs while the l0 matmuls still stream.
  - NormAct: h copied PSUM->SBUF bf16 (ACT), squares + adds + scale-muls
    on DVE bf16 2x, one batched Sqrt + Sigmoids per block.
  - Linear2: stationary W2 tile [v,128w], moving g -> out^T in PSUM,
    emitted l1, l2 first, l0 last (l0 needs the longest scale chain);
    copied to SBUF bf16 (mostly DVE), DMA'd out feature-major; host
    re-interleaves and upcasts to f32.

hsb row map (mid irreps): l1: rows 0..12 (kv*3+j), l2: rows 12..22
(kv*5+j), l0: rows 22..30 (kv).  out^T ft map (dram): l0: 0..4 (wt),
l1: 4..10 (ko*3+j), l2: 10..15 (j).
"""

import math
from contextlib import ExitStack

import numpy as np
import ml_dtypes

import concourse.bass as bass
import concourse.tile as tile
from concourse import bacc, mybir
from concourse.bass_utils import run_bass_kernel_spmd

F32 = mybir.dt.float32
BF16 = mybir.dt.bfloat16
AF = mybir.ActivationFunctionType
ALU = mybir.AluOpType

BF = ml_dtypes.bfloat16

N_CORES = 8
N_TOTAL = 32768
N_CORE = N_TOTAL // N_CORES          # 4096
BLK = 512
NBLK = N_CORE // BLK                 # 8

# stage-3 out slices copied via ACT (rest via DVE); early-emitted slices so
# the next block's Sqrt is not queued behind late copies on the ACT engine
S3_ACT = {4, 5, 6, 7, 8}
# stage-3 emission order: l1, l2 first; l0 last
S3_ORDER = list(range(4, 15)) + list(range(0, 4))
# Linear1 PSUM tiles copied via DVE (rest via ACT)
L1_DVE_TILES = set()


def _build():
    nc = bacc.Bacc(
        "TRN2", target_bir_lowering=False, debug=False, num_devices=N_CORES
    )

    xt_d = nc.dram_tensor("xt", [128, 15, N_CORE], BF16, kind="ExternalInput").ap()
    w1l0_d = nc.dram_tensor("w1l0", [128, 4, 1024], BF16, kind="ExternalInput").ap()
    w1l1_d = nc.dram_tensor("w1l1", [128, 2, 512], BF16, kind="ExternalInput").ap()
    w1l2_d = nc.dram_tensor("w1l2", [128, 256], BF16, kind="ExternalInput").ap()
    w2l0_d = nc.dram_tensor("w2l0", [128, 8, 512], BF16, kind="ExternalInput").ap()
    w2l1_d = nc.dram_tensor("w2l1", [128, 4, 256], BF16, kind="ExternalInput").ap()
    w2l2_d = nc.dram_tensor("w2l2", [128, 2, 128], BF16, kind="ExternalInput").ap()
    out_d = nc.dram_tensor("out", [128, 15, N_CORE], BF16, kind="ExternalOutput").ap()

    with tile.TileContext(nc) as tc, ExitStack() as ctx:
        consts = ctx.enter_context(tc.tile_pool(name="consts", bufs=1))
        sb = ctx.enter_context(tc.tile_pool(name="sb", bufs=1))
        ps = ctx.enter_context(tc.tile_pool(name="ps", bufs=1, space="PSUM"))

        w1l0 = consts.tile([128, 4, 1024], BF16)
        w1l1 = consts.tile([128, 2, 512], BF16)
        w1l2 = consts.tile([128, 256], BF16)
        w2l0 = consts.tile([128, 8, 512], BF16)
        w2l1 = consts.tile([128, 4, 256], BF16)
        w2l2 = consts.tile([128, 2, 128], BF16)
        # preload both ACT table sets (sigmoid first, sqrt last so block 0's
        # Sqrt finds its set resident) while the first DMAs stream
        dummy = consts.tile([128, 1], BF16)
        nc.vector.memset(dummy, 1.0)
        nc.scalar.activation(out=dummy, in_=dummy, func=AF.Sigmoid)
        nc.scalar.activation(out=dummy, in_=dummy, func=AF.Sqrt)

        def l1_slice(s, xt):
            """Accumulation list for mid slice s (hsb row s)."""
            if s < 12:
                kv, j = divmod(s, 3)
                return [
                    (w1l1[:, ki, kv * 128 : (kv + 1) * 128], xt[:, 4 + ki * 3 + j, :])
                    for ki in range(2)
                ]
            if s < 22:
                kv, j = divmod(s - 12, 5)
                return [(w1l2[:, kv * 128 : (kv + 1) * 128], xt[:, 10 + j, :])]
            kv = s - 22
            return [
                (w1l0[:, ki, kv * 128 : (kv + 1) * 128], xt[:, ki, :])
                for ki in range(4)
            ]

        def s3_slice(ft, hsb):
            if ft < 4:
                wt = ft
                return [
                    (w2l0[:, kv, wt * 128 : (wt + 1) * 128], hsb[:, 22 + kv, :])
                    for kv in range(8)
                ]
            if ft < 10:
                ko, j = divmod(ft - 4, 3)
                return [
                    (w2l1[:, kv, ko * 128 : (ko + 1) * 128], hsb[:, kv * 3 + j, :])
                    for kv in range(4)
                ]
            j = ft - 10
            return [(w2l2[:, kv, :], hsb[:, 12 + kv * 5 + j, :]) for kv in range(2)]

        def emit_l1(b):
            """DMA xt(b), Linear1 MMs + PSUM->SBUF copies + squares.
            Returns the per-block tile state."""
            st = {}
            st["xt"] = xt = sb.tile([128, 15, BLK], BF16, name="xt", tag="xt", bufs=2)
            nsl = slice(b * BLK, (b + 1) * BLK)
            # l1 rows first: the first Linear1 matmuls need only rows 4..10
            if b == 0:
                # interleave weight DMAs so the first matmuls start ASAP
                nc.sync.dma_start(out=w1l1, in_=w1l1_d)
                nc.sync.dma_start(out=xt[:, 4:10, :], in_=xt_d[:, 4:10, nsl])
                nc.sync.dma_start(out=w1l2, in_=w1l2_d)
                nc.sync.dma_start(out=xt[:, 10:15, :], in_=xt_d[:, 10:15, nsl])
                nc.sync.dma_start(out=w1l0, in_=w1l0_d)
                nc.sync.dma_start(out=xt[:, 0:4, :], in_=xt_d[:, 0:4, nsl])
            else:
                nc.sync.dma_start(out=xt[:, 4:10, :], in_=xt_d[:, 4:10, nsl])
                nc.sync.dma_start(out=xt[:, 10:15, :], in_=xt_d[:, 10:15, nsl])
                nc.sync.dma_start(out=xt[:, 0:4, :], in_=xt_d[:, 0:4, nsl])
            st["hsb"] = hsb = sb.tile(
                [128, 30, BLK], BF16, name="hsb", tag="hsb", bufs=2
            )
            st["sqb"] = sqb = sb.tile(
                [128, 22, BLK], BF16, name="sqb", tag="sqb", bufs=1
            )
            st["nrm"] = sb.tile([128, 6, BLK], BF16, name="nrm", tag="nrm", bufs=2)
            st["absb"] = sb.tile([128, 8, BLK], BF16, name="absb", tag="absb", bufs=2)
            st["negb"] = sb.tile([128, 8, BLK], BF16, name="negb", tag="negb", bufs=1)
            st["outsb"] = sb.tile(
                [128, 15, BLK], BF16, name="outsb", tag="outsb", bufs=2
            )
            hsb, sqb, nrm = st["hsb"], st["sqb"], st["nrm"]
            absb, negb = st["absb"], st["negb"]
            sq_done = 0
            for t in range(10):
                hm = ps.tile([128, 3, BLK], F32, name="hm", tag="hA", bufs=2)
                for r in range(3):
                    mms = l1_slice(t * 3 + r, xt)
                    for k, (w_ap, x_ap) in enumerate(mms):
                        nc.tensor.matmul(
                            hm[:, r, :], w_ap, x_ap,
                            start=(k == 0), stop=(k == len(mms) - 1),
                        )
                if t in L1_DVE_TILES:
                    nc.vector.tensor_copy(
                        out=hsb[:, t * 3 : (t + 1) * 3, :], in_=hm
                    )
                else:
                    nc.scalar.activation(
                        out=hsb[:, t * 3 : (t + 1) * 3, :], in_=hm, func=AF.Copy
                    )
                # squares (l1/l2 rows only, i.e. rows < 22) as rows land
                avail = min((t + 1) * 3, 22)
                if avail - sq_done >= 6 or (avail == 22 and avail > sq_done):
                    nc.vector.tensor_mul(
                        sqb[:, sq_done:avail, :],
                        hsb[:, sq_done:avail, :],
                        hsb[:, sq_done:avail, :],
                    )
                    sq_done = avail
                    if sq_done == 12:
                        # nsq l1 -> nrm[0:4] as soon as its squares land
                        v3 = sqb[:, 0:12, :].rearrange("p (k j) n -> p k j n", j=3)
                        nc.vector.tensor_add(
                            nrm[:, 0:4, :], v3[:, :, 0, :], v3[:, :, 1, :]
                        )
                        nc.vector.tensor_add(
                            nrm[:, 0:4, :], nrm[:, 0:4, :], v3[:, :, 2, :]
                        )
                    elif sq_done == 22:
                        v5 = sqb[:, 12:22, :].rearrange("p (k j) n -> p k j n", j=5)
                        nc.vector.tensor_add(
                            nrm[:, 4:6, :], v5[:, :, 0, :], v5[:, :, 1, :]
                        )
                        for j in (2, 3, 4):
                            nc.vector.tensor_add(
                                nrm[:, 4:6, :], nrm[:, 4:6, :], v5[:, :, j, :]
                            )
            # |h| for l0: max(-h, h) — emitted here so it runs well before
            # the sigmoid phase needs it
            nc.vector.tensor_scalar_mul(negb, hsb[:, 22:30, :], -1.0)
            nc.vector.tensor_tensor(
                out=absb, in0=hsb[:, 22:30, :], in1=negb, op=ALU.max
            )
            return st

        def emit_normact(st):
            hsb, nrm, absb = st["hsb"], st["nrm"], st["absb"]
            # n = sqrt(nsq), split by irrep so the l1 chain starts as soon as
            # its adds are done (sqrt table set; both before any Sigmoid)
            nc.scalar.activation(out=nrm[:, 0:4, :], in_=nrm[:, 0:4, :], func=AF.Sqrt)
            nc.scalar.activation(out=nrm[:, 4:6, :], in_=nrm[:, 4:6, :], func=AF.Sqrt)
            # scales = sigmoid(.), split so each gmul waits only on its slice
            nc.scalar.activation(out=nrm[:, 0:4, :], in_=nrm[:, 0:4, :], func=AF.Sigmoid)
            # g = h * scale (in place on hsb); l1 first (feeds stage-3 head)
            h3 = hsb[:, 0:12, :].rearrange("p (k j) n -> p k j n", j=3)
            nc.vector.tensor_mul(
                h3, h3,
                nrm[:, 0:4, :].unsqueeze(2).broadcast_to([128, 4, 3, BLK]),
            )
            nc.scalar.activation(out=nrm[:, 4:6, :], in_=nrm[:, 4:6, :], func=AF.Sigmoid)
            h5 = hsb[:, 12:22, :].rearrange("p (k j) n -> p k j n", j=5)
            nc.vector.tensor_mul(
                h5, h5,
                nrm[:, 4:6, :].unsqueeze(2).broadcast_to([128, 2, 5, BLK]),
            )
            nc.scalar.activation(out=absb, in_=absb, func=AF.Sigmoid)
            nc.vector.tensor_mul(hsb[:, 22:30, :], hsb[:, 22:30, :], absb)

        def emit_s3(b, st):
            hsb, outsb = st["hsb"], st["outsb"]
            nsl = slice(b * BLK, (b + 1) * BLK)
            for ft in S3_ORDER:
                q = ps.tile([128, BLK], F32, name="q", tag="hB", bufs=2)
                mms = s3_slice(ft, hsb)
                for k, (w_ap, g_ap) in enumerate(mms):
                    nc.tensor.matmul(
                        q, w_ap, g_ap, start=(k == 0), stop=(k == len(mms) - 1)
                    )
                if ft in S3_ACT:
                    nc.scalar.activation(out=outsb[:, ft, :], in_=q, func=AF.Copy)
                else:
                    nc.vector.tensor_copy(out=outsb[:, ft, :], in_=q)
                if ft == 9:
                    nc.sync.dma_start(
                        out=out_d[:, 4:10, nsl], in_=outsb[:, 4:10, :]
                    )
                elif ft == 14:
                    nc.sync.dma_start(
                        out=out_d[:, 10:15, nsl], in_=outsb[:, 10:15, :]
                    )
            nc.sync.dma_start(out=out_d[:, 0:4, nsl], in_=outsb[:, 0:4, :])

        # software-pipelined emission: PE always has L1(b+1) queued while
        # block b's norm chain completes, then drains s3(b).
        st = emit_l1(0)
        nc.sync.dma_start(out=w2l0, in_=w2l0_d)
        nc.sync.dma_start(out=w2l1, in_=w2l1_d)
        nc.sync.dma_start(out=w2l2, in_=w2l2_d)
        for b in range(NBLK):
            emit_normact(st)
            nxt = emit_l1(b + 1) if b + 1 < NBLK else None
            emit_s3(b, st)
            st = nxt

    nc.compile()
    return nc


_NC_CACHE = None


def _get_nc():
    global _NC_CACHE
    if _NC_CACHE is None:
        _NC_CACHE = _build()
    return _NC_CACHE


def _prep_weights(w1_l0, w1_l1, w1_l2, w2_l0, w2_l1, w2_l2):
    def t(w, scale, kt):
        w = np.asarray(w, np.float32) / scale
        if kt == 1:
            return np.ascontiguousarray(w.astype(BF))
        return np.ascontiguousarray(
            w.reshape(kt, 128, w.shape[1]).transpose(1, 0, 2).astype(BF)
        )

    return {
        "w1l0": t(w1_l0, math.sqrt(512.0), 4),
        "w1l1": t(w1_l1, math.sqrt(256.0), 2),
        "w1l2": t(w1_l2, math.sqrt(128.0), 1),
        "w2l0": t(w2_l0, math.sqrt(1024.0), 8),
        "w2l1": t(w2_l1, math.sqrt(512.0), 4),
        "w2l2": t(w2_l2, math.sqrt(256.0), 2),
    }


def _prep_x(x):
    """[32768, 1920] f32 -> [8, 128, 15, 4096] bf16, feature-major tiles."""
    xc = np.asarray(x, np.float32).reshape(N_CORES, N_CORE, 1920)
    xt = np.empty((N_CORES, 128, 15, N_CORE), dtype=BF)
    l0 = xc[:, :, 0:512].reshape(N_CORES, N_CORE, 4, 128)
    xt[:, :, 0:4, :] = l0.transpose(0, 3, 2, 1).astype(BF)
    l1 = xc[:, :, 512:1280].reshape(N_CORES, N_CORE, 2, 128, 3)
    xt[:, :, 4:10, :] = (
        l1.transpose(0, 3, 2, 4, 1).reshape(N_CORES, 128, 6, N_CORE).astype(BF)
    )
    l2 = xc[:, :, 1280:1920].reshape(N_CORES, N_CORE, 128, 5)
    xt[:, :, 10:15, :] = l2.transpose(0, 2, 3, 1).astype(BF)
    return xt


def _make_in_maps(x, w1_l0, w1_l1, w1_l2, w2_l0, w2_l1, w2_l2):
    ws = _prep_weights(w1_l0, w1_l1, w1_l2, w2_l0, w2_l1, w2_l2)
    xt = _prep_x(x)
    return [
        {"xt": np.ascontiguousarray(xt[c]), **ws} for c in range(N_CORES)
    ]


def _postprocess(results):
    """per-core out [128, 15, 4096] bf16 -> [32768, 1920] f32."""
    o = np.stack([np.asarray(results[c]["out"]) for c in range(N_CORES)])
    o = o.astype(np.float32)
    out = np.empty((N_CORES, N_CORE, 1920), np.float32)
    out[:, :, 0:512] = (
        o[:, :, 0:4, :].transpose(0, 3, 2, 1).reshape(N_CORES, N_CORE, 512)
    )
    out[:, :, 512:1280] = (
        o[:, :, 4:10, :]
        .reshape(N_CORES, 128, 2, 3, N_CORE)
        .transpose(0, 4, 2, 1, 3)
        .reshape(N_CORES, N_CORE, 768)
    )
    out[:, :, 1280:1920] = (
        o[:, :, 10:15, :].transpose(0, 3, 1, 2).reshape(N_CORES, N_CORE, 640)
    )
    return np.ascontiguousarray(out.reshape(N_TOTAL, 1920))


def kernel(x, w1_l0, w1_l1, w1_l2, w2_l0, w2_l1, w2_l2):
    nc = _get_nc()
    in_maps = _make_in_maps(x, w1_l0, w1_l1, w1_l2, w2_l0, w2_l1, w2_l2)
    res = run_bass_kernel_spmd(nc, in_maps, list(range(N_CORES))).results
    return _postprocess(res)
